# revision 54
# baseline (speedup 1.0000x reference)
"""Windowed (Swin-style) multi-head attention on 8 TRN2 NeuronCores.

Data-parallel: 256 independent windows -> 32 per core. Per window:
  qkv = x @ w_qkv ; per-head attn = softmax(q k^T * scale + bias) ; out = (attn v) @ w_proj + b_proj

Device-side layout strategy (all matmuls contract over the partition dim):
  - host pre-transposes x to channel-major xT[c, tok] so qT/kT are produced
    feature-major (ready to be score-matmul operands) and v token-major.
  - scores are computed TRANSPOSED, S^T[k, q] (lhsT = kT tile, rhs = qT), so
    softmax normalization runs over the partition axis:
      exp via ScalarE (scale folded in), * exp(bias) via VectorE,
      column-sums via ones-block matmul on TensorE (32 replicated rows at
      32-aligned partitions), reciprocal_approx_fast on VectorE, broadcast
      back to feature rows via an indicator matmul.
  - avT[f, q] = v-contracted matmul accumulated over k tiles; normalized
    avT is exactly the lhsT the projection matmul needs. b_proj is added
    (pre-broadcast on host) during the PSUM->SBUF output copy.
Matmul operands are bf16 (full-rate PE, fp32 PSUM accumulation); inputs are
rounded to bf16 on the host so they can be DMA'd directly.
"""

import sys

for _p in ("/opt/trn_rl_repo",):
    if _p not in sys.path:
        sys.path.insert(0, _p)

import ml_dtypes
import numpy as np
from contextlib import ExitStack

import concourse.bass as bass
import concourse.bacc as bacc
import concourse.mybir as mybir
from concourse import tile
from concourse.bass_utils import run_bass_kernel_spmd

NCORES = 8
BS = 256
W = BS // NCORES  # windows per core
N = 256           # tokens per window
DIM = 512
NH = 8
HD = 64
SCALE = HD ** -0.5
F32 = mybir.dt.float32
BF16 = mybir.dt.bfloat16
FP8 = mybir.dt.float8e4
NPBF = ml_dtypes.bfloat16
NPF8 = ml_dtypes.float8_e4m3fn
EXP = mybir.ActivationFunctionType.Exp
COPY = mybir.ActivationFunctionType.Copy
DR = mybir.MatmulPerfMode.DoubleRow


def build(w_count=W, b_zero=False):
    nc = bacc.Bacc(None, target_bir_lowering=False)
    xt = nc.declare_dram_parameter("xt", [w_count, DIM, N], BF16, False)
    wqk = nc.declare_dram_parameter("wqk", [DIM, 2 * DIM], BF16, False)
    wv = nc.declare_dram_parameter("wv", [DIM, DIM], BF16, False)
    wp = nc.declare_dram_parameter("wp", [DIM, DIM], BF16, False)
    brep = nc.declare_dram_parameter("brep", [128, DIM], F32, False)
    ebt = nc.declare_dram_parameter("ebt", [128, 2 * NH * N], BF16, False)
    ones8 = nc.declare_dram_parameter("ones8", [128, 2 * 128], FP8, False)
    out = nc.declare_dram_parameter("out", [w_count, N, DIM], BF16, True)

    with ExitStack() as ctx:
        tc = ctx.enter_context(tile.TileContext(nc))
        const = ctx.enter_context(tc.tile_pool(name="const", bufs=1))
        p_xt = ctx.enter_context(tc.tile_pool(name="xt", bufs=4))
        p_qk = ctx.enter_context(tc.tile_pool(name="qk", bufs=4))
        p_v = ctx.enter_context(tc.tile_pool(name="v", bufs=3))
        p_p = ctx.enter_context(tc.tile_pool(name="pp", bufs=4))
        p_p8 = ctx.enter_context(tc.tile_pool(name="pp8", bufs=4))
        p_e = ctx.enter_context(tc.tile_pool(name="te", bufs=6))
        p_bc = ctx.enter_context(tc.tile_pool(name="bc", bufs=4))
        p_av = ctx.enter_context(tc.tile_pool(name="av", bufs=4))
        p_rs = ctx.enter_context(tc.tile_pool(name="rs", bufs=3))
        p_o = ctx.enter_context(tc.tile_pool(name="os", bufs=6))
        ps = ctx.enter_context(tc.tile_pool(name="ps", bufs=2, space="PSUM"))
        psp = ctx.enter_context(tc.tile_pool(name="psp", bufs=2, space="PSUM"))
        ps2 = ctx.enter_context(tc.tile_pool(name="ps2", bufs=2, space="PSUM"))

        # wqk + ones on the sync queue (ahead of the xT loads, so the first
        # qkT can start ASAP); the later-needed constants go via gpsimd so
        # they never delay the xT stream.
        # wqk split per c-tile so the very first qkT matmul only waits for
        # one quarter of it; later-needed constants go via gpsimd (ebt
        # first: scores consume it before the now-deferred v phase).
        wqk_s = const.tile([128, 4, 2 * DIM], BF16)
        wqk_r = wqk.ap().rearrange("(t p) f -> p t f", p=128)
        # wqk rides the scalar-hosted queue in parallel with xt on sync;
        # pair 0 below runs ct-major so compute starts once ct0 arrives;
        # ct0 split in half so the first ft-group matmuls start sooner.
        nc.scalar.dma_start(wqk_s[:, 0, 0:DIM], wqk_r[:, 0, 0:DIM])
        nc.scalar.dma_start(wqk_s[:, 0, DIM:], wqk_r[:, 0, DIM:])
        for ct in range(1, 4):
            nc.scalar.dma_start(wqk_s[:, ct, :], wqk_r[:, ct, :])

        xt_ap0 = xt.ap()
        xt0 = p_xt.tile([128, 4, 2, N], BF16, tag="xt", name="xt_boot")
        for ct in range(4):
            nc.sync.dma_start(xt0[:, ct, 0, :],
                              xt_ap0[0, 128 * ct:128 * (ct + 1), :])
            nc.gpsimd.dma_start(xt0[:, ct, 1, :],
                                xt_ap0[1, 128 * ct:128 * (ct + 1), :])
        oc_s = const.tile([128, 2, 128], FP8)
        nc.gpsimd.dma_start(
            oc_s[:].rearrange("p a b -> p (a b)"), ones8.ap()
        )
        eb_s = const.tile([128, 2 * NH * N], BF16)
        nc.gpsimd.dma_start(eb_s[:], ebt.ap())
        wv_s = const.tile([128, 4, DIM], BF16)
        nc.gpsimd.dma_start(wv_s[:], wv.ap().rearrange("(t p) f -> p t f", p=128))
        wp_s = const.tile([128, 4, DIM], BF16)
        nc.gpsimd.dma_start(wp_s[:], wp.ap().rearrange("(t p) f -> p t f", p=128))
        br_s = const.tile([128, DIM], F32)
        nc.gpsimd.dma_start(br_s[:], brep.ap())

        # zero-padded K storage (3 persistent buffers rotated by pair):
        # slot s=0 holds [kE; 0], slot s=1 holds [0; kO]. Score matmuls can
        # then run as full 128-row tile configs against the unpadded q tile
        # (the other parity's q rows are killed by the zeros), eliminating
        # the ~106ns PE array drain on every 64<->128-row config switch.
        kpads = []
        for b in range(3):
            t = const.tile([128, 4, 2, 2, N], BF16, tag=f"kpad{b}",
                           name=f"kpad{b}")
            nc.vector.memset(t[64:128, :, :, 0, :], 0.0)
            nc.vector.memset(t[0:64, :, :, 1, :], 0.0)
            kpads.append(t)
        # zero-padded V storage, same trick for the avT matmuls: head h's
        # slot [p, wi, kt, h%2, h//2, 128] holds its 64 features in the
        # (h%2)*64 column half, zeros elsewhere, so every avT matmul is a
        # full (128,128) tile accumulating into full 128 psum partitions
        # (even heads land on rows 0-63, odd on 64-127) with no
        # tile_position packing and no config switches.
        vpads = []
        for b in range(3):
            t = const.tile([128, 2, 2, 2, 4, 128], BF16, tag=f"vpad{b}",
                           name=f"vpad{b}")
            nc.vector.memset(t[:, :, :, 0, :, 64:128], 0.0)
            nc.vector.memset(t[:, :, :, 1, :, 0:64], 0.0)
            vpads.append(t)

        xt_ap = xt.ap()
        out_ap = out.ap()

        # pp column index for (head, ktile): per head-pair the layout is
        # (e_k0, e_k1, o_k0, o_k1); score matmuls are ISSUED interleaved
        # e_k0, o_k0, e_k1, o_k1 so adjacent matmuls hit disjoint PE row
        # groups (and rowsum/avT orderings hit disjoint col groups).
        def ppi(h, kt):
            return (h // 2) * 4 + (h % 2) * 2 + kt

        def load_xt(wp2):
            # load xT (channel-major) for both windows: [128, ct, win, tok]
            if wp2 == 0:
                return xt0  # loaded up front across two queues
            xt_s = p_xt.tile([128, 4, 2, N], BF16, tag="xt")
            for wl in range(2):
                nc.sync.dma_start(
                    xt_s[:, :, wl, :],
                    xt_ap[2 * wp2 + wl].rearrange("(t p) q -> p t q", p=128),
                )
            return xt_s

        def qk_phase(wp2, xt_s, kpad):
            # qkT[feat, (win tok)] batched over the window pair (N=512 keeps
            # LDWEIGHTS hidden behind the matmul)
            qk_s = p_qk.tile([128, 4, 2, N], BF16, tag="qk")
            if wp2 == 0:
                # boot pair runs ct-major with 8 concurrent PSUM groups
                # (borrowing the idle scores/rowsum pools) so the first
                # matmul only waits for the ct0 DMAs
                accs = [ps.tile([128, 512], F32, tag="ps", name=f"qb_{f}")
                        for f in range(2)]
                accs += [psp.tile([128, 512], F32, tag="scp", name=f"qb_{f}")
                         for f in range(2, 4)]
                accs += [ps2.tile([128, 512], F32, tag="rs2", name=f"qb_{f}")
                         for f in range(4, 6)]
                for ct in range(4):
                    for ft in range(6):
                        nc.tensor.matmul(
                            accs[ft][:],
                            wqk_s[:, ct, ft * 128:(ft + 1) * 128],
                            xt_s[:, ct, :, :],
                            start=(ct == 0),
                            stop=(ct == 3),
                        )
                for ft in range(6):
                    if ft < 4:
                        nc.scalar.activation(
                            qk_s[:, ft, :, :].rearrange("p a q -> p (a q)"),
                            accs[ft][:], COPY,
                        )
                    else:
                        hp = ft - 4
                        nc.scalar.activation(
                            kpad[0:64, hp, :, 0, :],
                            accs[ft][0:64, :].rearrange(
                                "p (a q) -> p a q", a=2), COPY,
                        )
                        nc.vector.tensor_copy(
                            kpad[64:128, hp, :, 1, :],
                            accs[ft][64:128, :].rearrange(
                                "p (a q) -> p a q", a=2),
                        )
                for ft in range(6, 8):
                    acc = ps.tile([128, 512], F32, tag="ps")
                    for ct in range(4):
                        nc.tensor.matmul(
                            acc[:],
                            wqk_s[:, ct, ft * 128:(ft + 1) * 128],
                            xt_s[:, ct, :, :],
                            start=(ct == 0),
                            stop=(ct == 3),
                        )
                    hp = ft - 4
                    nc.scalar.activation(
                        kpad[0:64, hp, :, 0, :],
                        acc[0:64, :].rearrange("p (a q) -> p a q", a=2), COPY,
                    )
                    nc.vector.tensor_copy(
                        kpad[64:128, hp, :, 1, :],
                        acc[64:128, :].rearrange("p (a q) -> p a q", a=2),
                    )
            else:
                for ft in range(8):
                    acc = ps.tile([128, 512], F32, tag="ps")
                    for ct in range(4):
                        nc.tensor.matmul(
                            acc[:],
                            wqk_s[:, ct, ft * 128:(ft + 1) * 128],
                            xt_s[:, ct, :, :],
                            start=(ct == 0),
                            stop=(ct == 3),
                        )
                    if ft < 4:
                        # alternate copy engines so the 2-bank qk rotation
                        # recycles faster
                        dst = qk_s[:, ft, :, :].rearrange("p a q -> p (a q)")
                        if ft % 2 == 0:
                            nc.scalar.activation(dst, acc[:], COPY)
                        else:
                            nc.vector.tensor_copy(dst, acc[:])
                    else:
                        # k-ft: split into the zero-padded parity slots,
                        # halves run concurrently on ACT and DVE
                        hp = ft - 4
                        srcE = acc[0:64, :].rearrange(
                            "p (a q) -> p a q", a=2)
                        srcO = acc[64:128, :].rearrange(
                            "p (a q) -> p a q", a=2)
                        if ft % 2 == 0:
                            nc.scalar.activation(
                                kpad[0:64, hp, :, 0, :], srcE, COPY)
                            nc.vector.tensor_copy(
                                kpad[64:128, hp, :, 1, :], srcO)
                        else:
                            nc.vector.tensor_copy(
                                kpad[0:64, hp, :, 0, :], srcE)
                            nc.scalar.activation(
                                kpad[64:128, hp, :, 1, :], srcO, COPY)
            return qk_s

        def phase_v(xt_s, vpad):
            # v[tok, feat] (token-major), per window, written into the
            # zero-padded per-head slots
            for wi in range(2):
                for kt in range(2):
                    acc = ps.tile([128, 512], F32, tag="ps")
                    for ct in range(4):
                        nc.tensor.matmul(
                            acc[:],
                            xt_s[:, ct, wi, kt * 128:(kt + 1) * 128],
                            wv_s[:, ct, :],
                            start=(ct == 0),
                            stop=(ct == 3),
                        )
                    av_src = acc[:].rearrange("p (hh b) -> p hh b", hh=4)
                    nc.scalar.activation(
                        vpad[:, wi, kt, 0, :, 0:64],
                        av_src[:, :, 0:64], COPY,
                    )
                    nc.vector.tensor_copy(
                        vpad[:, wi, kt, 1, :, 64:128],
                        av_src[:, :, 64:128],
                    )

        def phase_scores(qk_s, kpad, wi):
                # scores^T -> exp(scale*s) * exp(bias) -> pp; then rowsums
                pp_s = p_p.tile([128, 2 * NH * N], BF16, tag="pp",
                                name=f"pp_{wi}")
                # fp8 copy of pp (x 1/8) for the DoubleRow rowsum matmuls;
                # produced chunk-by-chunk on the idle gpsimd engine
                pp8_s = p_p8.tile([128, 2 * NH * N], FP8, tag="pp8",
                                  name=f"pp8_{wi}")
                for hp in range(4):
                    scpE = psp.tile([128, 2, N], F32, tag="scp")
                    scpO = psp.tile([128, 2, N], F32, tag="scp")
                    scp = [scpE, scpO]
                    for kt in range(2):
                        for s in range(2):
                            # full 128-row tile config: lhsT is the
                            # zero-padded parity slot, rhs the full q tile
                            nc.tensor.matmul(
                                scp[s][:, kt, :],
                                kpad[:, hp, wi, s,
                                     kt * 128:(kt + 1) * 128],
                                qk_s[:, hp, wi, :],
                                start=True,
                                stop=True,
                            )
                    te = p_e.tile([128, 2, 2 * N], BF16, tag="te",
                                  name=f"te_{wi}_{hp}")
                    for s in range(2):
                        nc.scalar.activation(
                            te[:, s, :],
                            scp[s][:].rearrange("p a q -> p (a q)"),
                            EXP, scale=SCALE,
                        )
                    off = hp * 4 * N
                    nc.vector.tensor_mul(
                        pp_s[:, off:off + 4 * N],
                        te[:].rearrange("p a q -> p (a q)"),
                        eb_s[:, off:off + 4 * N],
                    )
                    c0, c1 = hp * 4 * N, (hp + 1) * 4 * N
                    # pp8 is stored [p, kt, ap2, s, sub2, q] so each rowsum
                    # DoubleRow rhs [p, s, (sub2 q)] is one CONTIGUOUS 4N
                    # slice (strided matmul rhs APs are very slow); chunks
                    # alternate DVE/ACT (gpsimd is far too slow and its ops
                    # lock the shared SBUF port)
                    dst = pp8_s[:].rearrange(
                        "p (k a s b q) -> p k a s b q",
                        k=2, a=2, s=2, b=2)[:, :, hp // 2, :, hp % 2]
                    src = pp_s[:, c0:c1].rearrange(
                        "p (s k q) -> p k s q", s=2, k=2)
                    if hp % 2 == 0:
                        nc.vector.tensor_scalar_mul(dst, src, 0.03125)
                    else:
                        nc.scalar.activation(dst, src, COPY, scale=0.03125)
                # rowsums broadcast straight to feature rows: fp8 DoubleRow
                # ones-block matmuls. The packing dim carries the head
                # parity s (not kt): lhsT[:,0,m]=32 for m<64, lhsT[:,1,m]=32
                # for m>=64, so one instruction writes both heads' sums on
                # their own 64-partition groups (dst partition 0 -- the ISA
                # rejects DoubleRow dst offsets). Halves rowsum PE cycles.
                rs0 = ps2.tile([128, 2, N], F32, tag="rs2", name=f"rs0_{wi}")
                rs1 = ps2.tile([128, 2, N], F32, tag="rs2", name=f"rs1_{wi}")
                rsp = [rs0, rs1]
                for ap2 in range(2):
                    for kt in range(2):
                        # 512-col DoubleRow (both sub2 blocks per inst);
                        # rhs is a contiguous 4N slice of pp8
                        base = (kt * 2 + ap2) * 4 * N
                        nc.tensor.matmul(
                            rsp[ap2][:, :, :],
                            oc_s[:],
                            pp8_s[:, base:base + 4 * N].rearrange(
                                "p (s m) -> p s m", s=2),
                            start=(kt == 0),
                            stop=(kt == 1),
                            perf_mode=DR,
                        )
                return pp_s, rsp

        def phase_recip(wi, rsp):
                # reciprocal (fp32 fast-approx) per feature-tile pair;
                # result is directly the avT normalizer
                rb0 = p_rs.tile([128, 512], F32, tag="rb", name=f"rb0_{wi}")
                rb1 = p_rs.tile([128, 512], F32, tag="rb", name=f"rb1_{wi}")
                rbs = [rb0, rb1]
                for ap2 in range(2):
                    nc.vector.reciprocal_approx_fast(
                        rbs[ap2][:], rsp[ap2][:].rearrange("p a q -> p (a q)")
                    )
                return rbs

        def phase_av(vpad, wi, pp_s, rbs):
                # avT[f, q]: zero-padded full (128,128) matmuls; even head
                # lands on psum rows 0-63, odd on 64-127, accumulated in
                # one 4-instruction chain per (ap2, sub2) region
                av_s = p_av.tile([128, 4 * N], BF16, tag="av",
                                 name=f"av_{wi}")
                for ap2 in range(2):
                    aa = ps.tile([128, 512], F32, tag="work",
                                 name=f"aa_{wi}_{ap2}")
                    for sub2 in range(2):
                        ftl = 2 * ap2 + sub2
                        for sub in range(2):
                            h = 2 * ftl + sub
                            for kt in range(2):
                                nc.tensor.matmul(
                                    aa[:, sub2 * N:(sub2 + 1) * N],
                                    vpad[:, wi, kt, sub, ftl, :],
                                    pp_s[:, ppi(h, kt) * N:(ppi(h, kt) + 1) * N],
                                    start=(sub == 0 and kt == 0),
                                    stop=(sub == 1 and kt == 1),
                                )
                    nc.vector.tensor_mul(
                        av_s[:, ap2 * 512:(ap2 + 1) * 512],
                        aa[:],
                        rbs[ap2][:],
                    )
                return av_s

        def phase_proj(w0, wi, av_s):
                # projection; add b_proj during PSUM->SBUF copy; output DMA
                # on the gpsimd queue (keeps the sync queue free for loads)
                w = w0 + wi
                for qt in range(2):
                    oo = ps.tile([128, 512], F32, tag="work",
                                 name=f"oo_{wi}_{qt}")
                    for ftl in range(4):
                        nc.tensor.matmul(
                            oo[:],
                            av_s[:, ftl * N + qt * 128:ftl * N + qt * 128 + 128],
                            wp_s[:, ftl, :],
                            start=(ftl == 0),
                            stop=(ftl == 3),
                        )
                    o_s = p_o.tile([128, DIM], BF16, tag="os",
                                   name=f"os_{wi}_{qt}")
                    if b_zero and qt == 0:
                        nc.scalar.activation(o_s[:], oo[:], COPY)
                    elif b_zero:
                        nc.vector.tensor_copy(o_s[:], oo[:])
                    else:
                        nc.vector.tensor_add(o_s[:], oo[:], br_s[:])
                    nc.gpsimd.dma_start(
                        out_ap[w, qt * 128:(qt + 1) * 128, :], o_s[:]
                    )

        def pair_tail(st, v_filler_xt):
            # the full post-qk chain for pair `st`, with the NEXT pair's
            # independent V matmuls emitted between the score matmuls and
            # the rowsums to cover the exp->mul->pp8 latency with PE work
            pp0, rsp0 = phase_scores(st["qk_s"], st["kpad"], 0)
            pp1, rsp1 = phase_scores(st["qk_s"], st["kpad"], 1)
            if v_filler_xt is not None:
                phase_v(v_filler_xt, st["vpad_next"])
            rb0 = phase_recip(0, rsp0)
            av0 = phase_av(st["vpad"], 0, pp0, rb0)
            phase_proj(st["w0"], 0, av0)
            rb1 = phase_recip(1, rsp1)
            av1 = phase_av(st["vpad"], 1, pp1, rb1)
            phase_proj(st["w0"], 1, av1)

        # software pipeline: emit pair k+1's qk block BEFORE pair k's
        # scores/av/proj tail so the PE runs long same-tile-config blocks
        # (each 64-row/128-row config switch costs a ~106ns array drain)
        prev = None
        for wp2 in range(w_count // 2):
            xt_s = load_xt(wp2)
            kpad = kpads[wp2 % 3]
            vpad = vpads[wp2 % 3]
            qk_s = qk_phase(wp2, xt_s, kpad)
            if prev is None:
                phase_v(xt_s, vpad)
                prev = {"w0": 2 * wp2, "qk_s": qk_s, "vpad": vpad,
                        "kpad": kpad}
                continue
            prev["vpad_next"] = vpad
            pair_tail(prev, xt_s)
            prev = {"w0": 2 * wp2, "qk_s": qk_s, "vpad": vpad,
                    "kpad": kpad}
        pair_tail(prev, None)

    nc.finalize()
    return nc


def _ones8_block():
    # [128, s(2), m(128)]: s=0 selects out rows 0-63 (even head), s=1 rows
    # 64-127 (odd head); value 32 cancels the 1/32 pp8 scale.
    o = np.zeros((128, 2, 128), np.float32)
    o[:, 0, :HD] = 32.0
    o[:, 1, HD:] = 32.0
    return np.ascontiguousarray(o.reshape(128, 256).astype(NPF8))


_NC_CACHE = {}


def _get_nc(w_count, b_zero):
    key = (w_count, b_zero)
    if key not in _NC_CACHE:
        _NC_CACHE[key] = build(w_count, b_zero)
    return _NC_CACHE[key]


def _prep(inputs, w_count):
    x = np.asarray(inputs["x"], dtype=np.float32)
    noise = np.asarray(inputs["noise"], dtype=np.float32)
    ns = np.asarray(inputs["noise_strength"], dtype=np.float32)
    wqkv = np.asarray(inputs["w_qkv"], dtype=np.float32)
    wproj = np.asarray(inputs["w_proj"], dtype=np.float32)
    bproj = np.asarray(inputs["b_proj"], dtype=np.float32)
    bt = np.asarray(inputs["bias_table"], dtype=np.float32)
    ri = np.asarray(inputs["rel_index"])

    xe = x + noise * ns                                     # [BS, N, DIM]
    xt = np.ascontiguousarray(xe.transpose(0, 2, 1).astype(NPBF))
    eb = np.exp(bt[ri])                                     # [q, k, h]
    ebT = eb.transpose(2, 1, 0)                             # [h, k, q]
    ebt = np.zeros((128, 2 * NH, N), np.float32)
    for h in range(NH):
        for kt in range(2):
            i = (h // 2) * 4 + (h % 2) * 2 + kt
            ebt[:, i, :] = ebT[h, kt * 128:(kt + 1) * 128, :]
    ebt = np.ascontiguousarray(ebt.reshape(128, 2 * NH * N).astype(NPBF))
    common = {
        "wqk": np.ascontiguousarray(wqkv[:, : 2 * DIM].astype(NPBF)),
        "wv": np.ascontiguousarray(wqkv[:, 2 * DIM:].astype(NPBF)),
        "wp": np.ascontiguousarray(wproj.astype(NPBF)),
        "brep": np.ascontiguousarray(
            np.broadcast_to(bproj.reshape(1, DIM), (128, DIM)).astype(np.float32)
        ),
        "ebt": ebt,
        "ones8": _ones8_block(),
    }
    in_maps = []
    for i in range(NCORES):
        m = dict(common)
        m["xt"] = np.ascontiguousarray(xt[i * w_count:(i + 1) * w_count])
        in_maps.append(m)
    return in_maps


def _run(inputs, w_count=W, trace=False, trace_cores=None):
    b_zero = not np.any(np.asarray(inputs["b_proj"], dtype=np.float32))
    nc = _get_nc(w_count, b_zero)
    in_maps = _prep(inputs, w_count)
    kw = {}
    if trace_cores is not None:
        kw["trace_cores"] = trace_cores
    res = run_bass_kernel_spmd(
        nc, in_maps, core_ids=list(range(NCORES)), trace=trace, **kw
    )
    full = np.concatenate(
        [res.results[i]["out"].astype(np.float32) for i in range(NCORES)], axis=0
    )
    return full, res


def kernel(**inputs):
    out, _ = _run(inputs, W, trace=False)
    return out


def kernel_profiled(inputs, w_count=W, trace_cores=None):
    out, res = _run(inputs, w_count, trace=True, trace_cores=trace_cores)
    return out, res



# revision 55
# speedup vs baseline: 1.1039x; 1.1039x over previous
"""Windowed (Swin-style) multi-head attention on 8 TRN2 NeuronCores.

Data-parallel: 256 independent windows -> 32 per core. Per window:
  qkv = x @ w_qkv ; per-head attn = softmax(q k^T * scale + bias) ; out = (attn v) @ w_proj + b_proj

Device-side layout strategy (all matmuls contract over the partition dim):
  - host pre-transposes x to channel-major xT[c, tok] so qT/kT are produced
    feature-major (ready to be score-matmul operands) and v token-major.
  - scores are computed TRANSPOSED, S^T[k, q] (lhsT = kT tile, rhs = qT), so
    softmax normalization runs over the partition axis:
      exp via ScalarE (scale folded in), * exp(bias) via VectorE,
      column-sums via ones-block matmul on TensorE (32 replicated rows at
      32-aligned partitions), reciprocal_approx_fast on VectorE, broadcast
      back to feature rows via an indicator matmul.
  - avT[f, q] = v-contracted matmul accumulated over k tiles; normalized
    avT is exactly the lhsT the projection matmul needs. b_proj is added
    (pre-broadcast on host) during the PSUM->SBUF output copy.
Matmul operands are bf16 (full-rate PE, fp32 PSUM accumulation); inputs are
rounded to bf16 on the host so they can be DMA'd directly.
"""

import sys

for _p in ("/opt/trn_rl_repo",):
    if _p not in sys.path:
        sys.path.insert(0, _p)

import ml_dtypes
import numpy as np
from contextlib import ExitStack

import concourse.bass as bass
import concourse.bacc as bacc
import concourse.mybir as mybir
from concourse import tile
from concourse.bass_utils import run_bass_kernel_spmd

NCORES = 8
BS = 256
W = BS // NCORES  # windows per core
N = 256           # tokens per window
DIM = 512
NH = 8
HD = 64
SCALE = HD ** -0.5
F32 = mybir.dt.float32
BF16 = mybir.dt.bfloat16
FP8 = mybir.dt.float8e4
NPBF = ml_dtypes.bfloat16
NPF8 = ml_dtypes.float8_e4m3fn
EXP = mybir.ActivationFunctionType.Exp
COPY = mybir.ActivationFunctionType.Copy
DR = mybir.MatmulPerfMode.DoubleRow


def build(w_count=W, b_zero=False):
    nc = bacc.Bacc(None, target_bir_lowering=False)
    xt = nc.declare_dram_parameter("xt", [w_count, DIM, N], BF16, False)
    wqk = nc.declare_dram_parameter("wqk", [DIM, 2 * DIM], BF16, False)
    wv = nc.declare_dram_parameter("wv", [DIM, DIM], BF16, False)
    wp = nc.declare_dram_parameter("wp", [DIM, DIM], BF16, False)
    brep = nc.declare_dram_parameter("brep", [128, DIM], F32, False)
    ebt = nc.declare_dram_parameter("ebt", [128, 2 * NH * N], BF16, False)
    ones8 = nc.declare_dram_parameter("ones8", [128, 2 * 128], FP8, False)
    out = nc.declare_dram_parameter("out", [w_count, N, DIM], BF16, True)

    with ExitStack() as ctx:
        tc = ctx.enter_context(tile.TileContext(nc))
        const = ctx.enter_context(tc.tile_pool(name="const", bufs=1))
        p_xt = ctx.enter_context(tc.tile_pool(name="xt", bufs=4))
        p_qk = ctx.enter_context(tc.tile_pool(name="qk", bufs=4))
        p_v = ctx.enter_context(tc.tile_pool(name="v", bufs=3))
        p_p = ctx.enter_context(tc.tile_pool(name="pp", bufs=4))
        p_p8 = ctx.enter_context(tc.tile_pool(name="pp8", bufs=4))
        p_e = ctx.enter_context(tc.tile_pool(name="te", bufs=6))
        p_bc = ctx.enter_context(tc.tile_pool(name="bc", bufs=4))
        p_av = ctx.enter_context(tc.tile_pool(name="av", bufs=4))
        p_rs = ctx.enter_context(tc.tile_pool(name="rs", bufs=3))
        p_o = ctx.enter_context(tc.tile_pool(name="os", bufs=6))
        ps = ctx.enter_context(tc.tile_pool(name="ps", bufs=2, space="PSUM"))
        psp = ctx.enter_context(tc.tile_pool(name="psp", bufs=2, space="PSUM"))
        ps2 = ctx.enter_context(tc.tile_pool(name="ps2", bufs=2, space="PSUM"))

        # wqk + ones on the sync queue (ahead of the xT loads, so the first
        # qkT can start ASAP); the later-needed constants go via gpsimd so
        # they never delay the xT stream.
        # wqk split per c-tile so the very first qkT matmul only waits for
        # one quarter of it; later-needed constants go via gpsimd (ebt
        # first: scores consume it before the now-deferred v phase).
        wqk_s = const.tile([128, 4, 2 * DIM], BF16)
        wqk_r = wqk.ap().rearrange("(t p) f -> p t f", p=128)
        # wqk rides the scalar-hosted queue in parallel with xt on sync;
        # pair 0 below runs ct-major so compute starts once ct0 arrives;
        # ct0 split in half so the first ft-group matmuls start sooner.
        nc.scalar.dma_start(wqk_s[:, 0, 0:DIM], wqk_r[:, 0, 0:DIM])
        nc.scalar.dma_start(wqk_s[:, 0, DIM:], wqk_r[:, 0, DIM:])
        for ct in range(1, 4):
            nc.scalar.dma_start(wqk_s[:, ct, :], wqk_r[:, ct, :])

        xt_ap0 = xt.ap()
        xt0 = p_xt.tile([128, 4, 2, N], BF16, tag="xt", name="xt_boot")
        for ct in range(4):
            nc.sync.dma_start(xt0[:, ct, 0, :],
                              xt_ap0[0, 128 * ct:128 * (ct + 1), :])
            nc.gpsimd.dma_start(xt0[:, ct, 1, :],
                                xt_ap0[1, 128 * ct:128 * (ct + 1), :])
        oc_s = const.tile([128, 2, 128], FP8)
        nc.gpsimd.dma_start(
            oc_s[:].rearrange("p a b -> p (a b)"), ones8.ap()
        )
        eb_s = const.tile([128, 2 * NH * N], BF16)
        nc.gpsimd.dma_start(eb_s[:], ebt.ap())
        wv_s = const.tile([128, 4, DIM], BF16)
        nc.gpsimd.dma_start(wv_s[:], wv.ap().rearrange("(t p) f -> p t f", p=128))
        wp_s = const.tile([128, 4, DIM], BF16)
        nc.gpsimd.dma_start(wp_s[:], wp.ap().rearrange("(t p) f -> p t f", p=128))
        br_s = const.tile([128, DIM], F32)
        nc.gpsimd.dma_start(br_s[:], brep.ap())

        # zero-padded K storage (3 persistent buffers rotated by pair):
        # slot s=0 holds [kE; 0], slot s=1 holds [0; kO]. Score matmuls can
        # then run as full 128-row tile configs against the unpadded q tile
        # (the other parity's q rows are killed by the zeros), eliminating
        # the ~106ns PE array drain on every 64<->128-row config switch.
        kpads = []
        for b in range(3):
            t = const.tile([128, 4, 2, 2, N], BF16, tag=f"kpad{b}",
                           name=f"kpad{b}")
            nc.vector.memset(t[64:128, :, :, 0, :], 0.0)
            nc.vector.memset(t[0:64, :, :, 1, :], 0.0)
            kpads.append(t)

        xt_ap = xt.ap()
        out_ap = out.ap()

        # pp column index for (head, ktile): per head-pair the layout is
        # (e_k0, e_k1, o_k0, o_k1); score matmuls are ISSUED interleaved
        # e_k0, o_k0, e_k1, o_k1 so adjacent matmuls hit disjoint PE row
        # groups (and rowsum/avT orderings hit disjoint col groups).
        def ppi(h, kt):
            return (h // 2) * 4 + (h % 2) * 2 + kt

        def load_xt(wp2):
            # load xT (channel-major) for both windows: [128, ct, win, tok]
            if wp2 == 0:
                return xt0  # loaded up front across two queues
            xt_s = p_xt.tile([128, 4, 2, N], BF16, tag="xt")
            for wl in range(2):
                nc.sync.dma_start(
                    xt_s[:, :, wl, :],
                    xt_ap[2 * wp2 + wl].rearrange("(t p) q -> p t q", p=128),
                )
            return xt_s

        def qk_phase(wp2, xt_s, kpad):
            # qkT[feat, (win tok)] batched over the window pair (N=512 keeps
            # LDWEIGHTS hidden behind the matmul)
            qk_s = p_qk.tile([128, 8, 2, N], BF16, tag="qk")
            if wp2 == 0:
                # boot pair runs ct-major with 8 concurrent PSUM groups
                # (borrowing the idle scores/rowsum pools) so the first
                # matmul only waits for the ct0 DMAs
                accs = [ps.tile([128, 512], F32, tag="ps", name=f"qb_{f}")
                        for f in range(2)]
                accs += [psp.tile([128, 512], F32, tag="scp", name=f"qb_{f}")
                         for f in range(2, 4)]
                accs += [ps2.tile([128, 512], F32, tag="rs2", name=f"qb_{f}")
                         for f in range(4, 6)]
                for ct in range(4):
                    for ft in range(6):
                        nc.tensor.matmul(
                            accs[ft][:],
                            wqk_s[:, ct, ft * 128:(ft + 1) * 128],
                            xt_s[:, ct, :, :],
                            start=(ct == 0),
                            stop=(ct == 3),
                        )
                for ft in range(6):
                    if ft < 4:
                        nc.scalar.activation(
                            qk_s[:, ft, :, :].rearrange("p a q -> p (a q)"),
                            accs[ft][:], COPY,
                        )
                    else:
                        hp = ft - 4
                        nc.scalar.activation(
                            kpad[0:64, hp, :, 0, :],
                            accs[ft][0:64, :].rearrange(
                                "p (a q) -> p a q", a=2), COPY,
                        )
                        nc.vector.tensor_copy(
                            kpad[64:128, hp, :, 1, :],
                            accs[ft][64:128, :].rearrange(
                                "p (a q) -> p a q", a=2),
                        )
                for ft in range(6, 8):
                    acc = ps.tile([128, 512], F32, tag="ps")
                    for ct in range(4):
                        nc.tensor.matmul(
                            acc[:],
                            wqk_s[:, ct, ft * 128:(ft + 1) * 128],
                            xt_s[:, ct, :, :],
                            start=(ct == 0),
                            stop=(ct == 3),
                        )
                    hp = ft - 4
                    nc.scalar.activation(
                        kpad[0:64, hp, :, 0, :],
                        acc[0:64, :].rearrange("p (a q) -> p a q", a=2), COPY,
                    )
                    nc.vector.tensor_copy(
                        kpad[64:128, hp, :, 1, :],
                        acc[64:128, :].rearrange("p (a q) -> p a q", a=2),
                    )
            else:
                for ft in range(8):
                    acc = ps.tile([128, 512], F32, tag="ps")
                    for ct in range(4):
                        nc.tensor.matmul(
                            acc[:],
                            wqk_s[:, ct, ft * 128:(ft + 1) * 128],
                            xt_s[:, ct, :, :],
                            start=(ct == 0),
                            stop=(ct == 3),
                        )
                    if ft < 4:
                        # alternate copy engines so the 2-bank qk rotation
                        # recycles faster
                        dst = qk_s[:, ft, :, :].rearrange("p a q -> p (a q)")
                        if ft % 2 == 0:
                            nc.scalar.activation(dst, acc[:], COPY)
                        else:
                            nc.vector.tensor_copy(dst, acc[:])
                    else:
                        # k-ft: split into the zero-padded parity slots,
                        # halves run concurrently on ACT and DVE
                        hp = ft - 4
                        srcE = acc[0:64, :].rearrange(
                            "p (a q) -> p a q", a=2)
                        srcO = acc[64:128, :].rearrange(
                            "p (a q) -> p a q", a=2)
                        if ft % 2 == 0:
                            nc.scalar.activation(
                                kpad[0:64, hp, :, 0, :], srcE, COPY)
                            nc.vector.tensor_copy(
                                kpad[64:128, hp, :, 1, :], srcO)
                        else:
                            nc.vector.tensor_copy(
                                kpad[0:64, hp, :, 0, :], srcE)
                            nc.scalar.activation(
                                kpad[64:128, hp, :, 1, :], srcO, COPY)
            return qk_s

        def phase_v(xt_s):
            # v[tok, feat] (token-major), per window
            v_s = p_v.tile([128, 2, 2, DIM], BF16, tag="v")
            for wi in range(2):
                for kt in range(2):
                    acc = ps.tile([128, 512], F32, tag="ps")
                    for ct in range(4):
                        nc.tensor.matmul(
                            acc[:],
                            xt_s[:, ct, wi, kt * 128:(kt + 1) * 128],
                            wv_s[:, ct, :],
                            start=(ct == 0),
                            stop=(ct == 3),
                        )
                    nc.vector.tensor_copy(
                        v_s[:, wi, kt, :], acc[:]
                    )
            return v_s

        def phase_scores(qk_s, kpad, wi):
                # scores^T -> exp(scale*s) * exp(bias) -> pp; then rowsums
                pp_s = p_p.tile([128, 2 * NH * N], BF16, tag="pp",
                                name=f"pp_{wi}")
                # fp8 copy of pp (x 1/8) for the DoubleRow rowsum matmuls;
                # produced chunk-by-chunk on the idle gpsimd engine
                pp8_s = p_p8.tile([128, 2 * NH * N], FP8, tag="pp8",
                                  name=f"pp8_{wi}")
                for hp in range(4):
                    scpE = psp.tile([128, 2, N], F32, tag="scp")
                    scpO = psp.tile([128, 2, N], F32, tag="scp")
                    scp = [scpE, scpO]
                    for kt in range(2):
                        for s in range(2):
                            # full 128-row tile config: lhsT is the
                            # zero-padded parity slot, rhs the full q tile
                            nc.tensor.matmul(
                                scp[s][:, kt, :],
                                kpad[:, hp, wi, s,
                                     kt * 128:(kt + 1) * 128],
                                qk_s[:, hp, wi, :],
                                start=True,
                                stop=True,
                            )
                    te = p_e.tile([128, 2, 2 * N], BF16, tag="te",
                                  name=f"te_{wi}_{hp}")
                    for s in range(2):
                        nc.scalar.activation(
                            te[:, s, :],
                            scp[s][:].rearrange("p a q -> p (a q)"),
                            EXP, scale=SCALE,
                        )
                    off = hp * 4 * N
                    nc.vector.tensor_mul(
                        pp_s[:, off:off + 4 * N],
                        te[:].rearrange("p a q -> p (a q)"),
                        eb_s[:, off:off + 4 * N],
                    )
                    c0, c1 = hp * 4 * N, (hp + 1) * 4 * N
                    # pp8 is stored [p, kt, ap2, s, sub2, q] so each rowsum
                    # DoubleRow rhs [p, s, (sub2 q)] is one CONTIGUOUS 4N
                    # slice (strided matmul rhs APs are very slow); chunks
                    # alternate DVE/ACT (gpsimd is far too slow and its ops
                    # lock the shared SBUF port)
                    dst = pp8_s[:].rearrange(
                        "p (k a s b q) -> p k a s b q",
                        k=2, a=2, s=2, b=2)[:, :, hp // 2, :, hp % 2]
                    src = pp_s[:, c0:c1].rearrange(
                        "p (s k q) -> p k s q", s=2, k=2)
                    if hp % 2 == 0:
                        nc.vector.tensor_scalar_mul(dst, src, 0.03125)
                    else:
                        nc.scalar.activation(dst, src, COPY, scale=0.03125)
                # rowsums broadcast straight to feature rows: fp8 DoubleRow
                # ones-block matmuls. The packing dim carries the head
                # parity s (not kt): lhsT[:,0,m]=32 for m<64, lhsT[:,1,m]=32
                # for m>=64, so one instruction writes both heads' sums on
                # their own 64-partition groups (dst partition 0 -- the ISA
                # rejects DoubleRow dst offsets). Halves rowsum PE cycles.
                rs0 = ps2.tile([128, 2, N], F32, tag="rs2", name=f"rs0_{wi}")
                rs1 = ps2.tile([128, 2, N], F32, tag="rs2", name=f"rs1_{wi}")
                rsp = [rs0, rs1]
                for ap2 in range(2):
                    for kt in range(2):
                        # 512-col DoubleRow (both sub2 blocks per inst);
                        # rhs is a contiguous 4N slice of pp8
                        base = (kt * 2 + ap2) * 4 * N
                        nc.tensor.matmul(
                            rsp[ap2][:, :, :],
                            oc_s[:],
                            pp8_s[:, base:base + 4 * N].rearrange(
                                "p (s m) -> p s m", s=2),
                            start=(kt == 0),
                            stop=(kt == 1),
                            perf_mode=DR,
                        )
                return pp_s, rsp

        def phase_recip(wi, rsp):
                # reciprocal (fp32 fast-approx) per feature-tile pair;
                # result is directly the avT normalizer
                rb0 = p_rs.tile([128, 512], F32, tag="rb", name=f"rb0_{wi}")
                rb1 = p_rs.tile([128, 512], F32, tag="rb", name=f"rb1_{wi}")
                rbs = [rb0, rb1]
                for ap2 in range(2):
                    nc.vector.reciprocal_approx_fast(
                        rbs[ap2][:], rsp[ap2][:].rearrange("p a q -> p (a q)")
                    )
                return rbs

        def phase_av(v_s, wi, pp_s, rbs):
                # avT[f, q]: head pairs in PE column groups; batched
                # normalize-mult per two feature tiles
                av_s = p_av.tile([128, 4 * N], BF16, tag="av",
                                 name=f"av_{wi}")
                for ap2 in range(2):
                    aa = ps.tile([128, 512], F32, tag="work",
                                 name=f"aa_{wi}_{ap2}")
                    for sub2 in range(2):
                        ftl = 2 * ap2 + sub2
                        for sub in range(2):
                            h = 2 * ftl + sub
                            for kt in range(2):
                                nc.tensor.matmul(
                                    aa[sub * HD:(sub + 1) * HD,
                                       sub2 * N:(sub2 + 1) * N],
                                    v_s[:, wi, kt, h * HD:(h + 1) * HD],
                                    pp_s[:, ppi(h, kt) * N:(ppi(h, kt) + 1) * N],
                                    start=(kt == 0),
                                    stop=(kt == 1),
                                    tile_position=(0, sub * HD),
                                )
                    nc.vector.tensor_mul(
                        av_s[:, ap2 * 512:(ap2 + 1) * 512],
                        aa[:],
                        rbs[ap2][:],
                    )
                return av_s

        def phase_proj(w0, wi, av_s):
                # projection; add b_proj during PSUM->SBUF copy; output DMA
                # on the gpsimd queue (keeps the sync queue free for loads)
                w = w0 + wi
                for qt in range(2):
                    oo = ps.tile([128, 512], F32, tag="work",
                                 name=f"oo_{wi}_{qt}")
                    for ftl in range(4):
                        nc.tensor.matmul(
                            oo[:],
                            av_s[:, ftl * N + qt * 128:ftl * N + qt * 128 + 128],
                            wp_s[:, ftl, :],
                            start=(ftl == 0),
                            stop=(ftl == 3),
                        )
                    o_s = p_o.tile([128, DIM], BF16, tag="os",
                                   name=f"os_{wi}_{qt}")
                    if b_zero and qt == 0:
                        nc.scalar.activation(o_s[:], oo[:], COPY)
                    elif b_zero:
                        nc.vector.tensor_copy(o_s[:], oo[:])
                    else:
                        nc.vector.tensor_add(o_s[:], oo[:], br_s[:])
                    nc.gpsimd.dma_start(
                        out_ap[w, qt * 128:(qt + 1) * 128, :], o_s[:]
                    )

        def pair_tail(st, v_filler_xt):
            # the full post-qk chain for pair `st`, with the NEXT pair's
            # independent V matmuls emitted between the score matmuls and
            # the rowsums to cover the exp->mul->pp8 latency with PE work
            pp0, rsp0 = phase_scores(st["qk_s"], st["kpad"], 0)
            pp1, rsp1 = phase_scores(st["qk_s"], st["kpad"], 1)
            v_next = phase_v(v_filler_xt) if v_filler_xt is not None else None
            rb0 = phase_recip(0, rsp0)
            av0 = phase_av(st["v_s"], 0, pp0, rb0)
            phase_proj(st["w0"], 0, av0)
            rb1 = phase_recip(1, rsp1)
            av1 = phase_av(st["v_s"], 1, pp1, rb1)
            phase_proj(st["w0"], 1, av1)
            return v_next

        # software pipeline: emit pair k+1's qk block BEFORE pair k's
        # scores/av/proj tail so the PE runs long same-tile-config blocks
        # (each 64-row/128-row config switch costs a ~106ns array drain)
        prev = None
        for wp2 in range(w_count // 2):
            xt_s = load_xt(wp2)
            kpad = kpads[wp2 % 3]
            qk_s = qk_phase(wp2, xt_s, kpad)
            if prev is None:
                v_s = phase_v(xt_s)
                prev = {"w0": 2 * wp2, "qk_s": qk_s, "v_s": v_s,
                        "kpad": kpad}
                continue
            v_s = pair_tail(prev, xt_s)
            prev = {"w0": 2 * wp2, "qk_s": qk_s, "v_s": v_s,
                    "kpad": kpad}
        pair_tail(prev, None)

    nc.finalize()
    return nc


def _ones8_block():
    # [128, s(2), m(128)]: s=0 selects out rows 0-63 (even head), s=1 rows
    # 64-127 (odd head); value 32 cancels the 1/32 pp8 scale.
    o = np.zeros((128, 2, 128), np.float32)
    o[:, 0, :HD] = 32.0
    o[:, 1, HD:] = 32.0
    return np.ascontiguousarray(o.reshape(128, 256).astype(NPF8))


_NC_CACHE = {}


def _get_nc(w_count, b_zero):
    key = (w_count, b_zero)
    if key not in _NC_CACHE:
        _NC_CACHE[key] = build(w_count, b_zero)
    return _NC_CACHE[key]


def _prep(inputs, w_count):
    x = np.asarray(inputs["x"], dtype=np.float32)
    noise = np.asarray(inputs["noise"], dtype=np.float32)
    ns = np.asarray(inputs["noise_strength"], dtype=np.float32)
    wqkv = np.asarray(inputs["w_qkv"], dtype=np.float32)
    wproj = np.asarray(inputs["w_proj"], dtype=np.float32)
    bproj = np.asarray(inputs["b_proj"], dtype=np.float32)
    bt = np.asarray(inputs["bias_table"], dtype=np.float32)
    ri = np.asarray(inputs["rel_index"])

    xe = x + noise * ns                                     # [BS, N, DIM]
    xt = np.ascontiguousarray(xe.transpose(0, 2, 1).astype(NPBF))
    eb = np.exp(bt[ri])                                     # [q, k, h]
    ebT = eb.transpose(2, 1, 0)                             # [h, k, q]
    ebt = np.zeros((128, 2 * NH, N), np.float32)
    for h in range(NH):
        for kt in range(2):
            i = (h // 2) * 4 + (h % 2) * 2 + kt
            ebt[:, i, :] = ebT[h, kt * 128:(kt + 1) * 128, :]
    ebt = np.ascontiguousarray(ebt.reshape(128, 2 * NH * N).astype(NPBF))
    common = {
        "wqk": np.ascontiguousarray(wqkv[:, : 2 * DIM].astype(NPBF)),
        "wv": np.ascontiguousarray(wqkv[:, 2 * DIM:].astype(NPBF)),
        "wp": np.ascontiguousarray(wproj.astype(NPBF)),
        "brep": np.ascontiguousarray(
            np.broadcast_to(bproj.reshape(1, DIM), (128, DIM)).astype(np.float32)
        ),
        "ebt": ebt,
        "ones8": _ones8_block(),
    }
    in_maps = []
    for i in range(NCORES):
        m = dict(common)
        m["xt"] = np.ascontiguousarray(xt[i * w_count:(i + 1) * w_count])
        in_maps.append(m)
    return in_maps


def _run(inputs, w_count=W, trace=False, trace_cores=None):
    b_zero = not np.any(np.asarray(inputs["b_proj"], dtype=np.float32))
    nc = _get_nc(w_count, b_zero)
    in_maps = _prep(inputs, w_count)
    kw = {}
    if trace_cores is not None:
        kw["trace_cores"] = trace_cores
    res = run_bass_kernel_spmd(
        nc, in_maps, core_ids=list(range(NCORES)), trace=trace, **kw
    )
    full = np.concatenate(
        [res.results[i]["out"].astype(np.float32) for i in range(NCORES)], axis=0
    )
    return full, res


def kernel(**inputs):
    out, _ = _run(inputs, W, trace=False)
    return out


def kernel_profiled(inputs, w_count=W, trace_cores=None):
    out, res = _run(inputs, w_count, trace=True, trace_cores=trace_cores)
    return out, res



# revision 56
# speedup vs baseline: 1.1106x; 1.0061x over previous
"""Windowed (Swin-style) multi-head attention on 8 TRN2 NeuronCores.

Data-parallel: 256 independent windows -> 32 per core. Per window:
  qkv = x @ w_qkv ; per-head attn = softmax(q k^T * scale + bias) ; out = (attn v) @ w_proj + b_proj

Device-side layout strategy (all matmuls contract over the partition dim):
  - host pre-transposes x to channel-major xT[c, tok] so qT/kT are produced
    feature-major (ready to be score-matmul operands) and v token-major.
  - scores are computed TRANSPOSED, S^T[k, q] (lhsT = kT tile, rhs = qT), so
    softmax normalization runs over the partition axis:
      exp via ScalarE (scale folded in), * exp(bias) via VectorE,
      column-sums via ones-block matmul on TensorE (32 replicated rows at
      32-aligned partitions), reciprocal_approx_fast on VectorE, broadcast
      back to feature rows via an indicator matmul.
  - avT[f, q] = v-contracted matmul accumulated over k tiles; normalized
    avT is exactly the lhsT the projection matmul needs. b_proj is added
    (pre-broadcast on host) during the PSUM->SBUF output copy.
Matmul operands are bf16 (full-rate PE, fp32 PSUM accumulation); inputs are
rounded to bf16 on the host so they can be DMA'd directly.
"""

import sys

for _p in ("/opt/trn_rl_repo",):
    if _p not in sys.path:
        sys.path.insert(0, _p)

import ml_dtypes
import numpy as np
from contextlib import ExitStack

import concourse.bass as bass
import concourse.bacc as bacc
import concourse.mybir as mybir
from concourse import tile
from concourse.bass_utils import run_bass_kernel_spmd

NCORES = 8
BS = 256
W = BS // NCORES  # windows per core
N = 256           # tokens per window
DIM = 512
NH = 8
HD = 64
SCALE = HD ** -0.5
F32 = mybir.dt.float32
BF16 = mybir.dt.bfloat16
FP8 = mybir.dt.float8e4
NPBF = ml_dtypes.bfloat16
NPF8 = ml_dtypes.float8_e4m3fn
EXP = mybir.ActivationFunctionType.Exp
COPY = mybir.ActivationFunctionType.Copy
DR = mybir.MatmulPerfMode.DoubleRow


def build(w_count=W, b_zero=False):
    nc = bacc.Bacc(None, target_bir_lowering=False)
    xt = nc.declare_dram_parameter("xt", [w_count, DIM, N], BF16, False)
    wqk = nc.declare_dram_parameter("wqk", [DIM, 2 * DIM], BF16, False)
    wv = nc.declare_dram_parameter("wv", [DIM, DIM], BF16, False)
    wp = nc.declare_dram_parameter("wp", [DIM, DIM], BF16, False)
    brep = nc.declare_dram_parameter("brep", [128, DIM], F32, False)
    ebt = nc.declare_dram_parameter("ebt", [128, 2 * NH * N], BF16, False)
    ones8 = nc.declare_dram_parameter("ones8", [128, 2 * 128], FP8, False)
    out = nc.declare_dram_parameter("out", [w_count, N, DIM], BF16, True)

    with ExitStack() as ctx:
        tc = ctx.enter_context(tile.TileContext(nc))
        const = ctx.enter_context(tc.tile_pool(name="const", bufs=1))
        p_xt = ctx.enter_context(tc.tile_pool(name="xt", bufs=4))
        p_qk = ctx.enter_context(tc.tile_pool(name="qk", bufs=4))
        p_v = ctx.enter_context(tc.tile_pool(name="v", bufs=3))
        p_p = ctx.enter_context(tc.tile_pool(name="pp", bufs=4))
        p_p8 = ctx.enter_context(tc.tile_pool(name="pp8", bufs=4))
        p_e = ctx.enter_context(tc.tile_pool(name="te", bufs=6))
        p_bc = ctx.enter_context(tc.tile_pool(name="bc", bufs=4))
        p_av = ctx.enter_context(tc.tile_pool(name="av", bufs=4))
        p_rs = ctx.enter_context(tc.tile_pool(name="rs", bufs=3))
        p_o = ctx.enter_context(tc.tile_pool(name="os", bufs=6))
        ps = ctx.enter_context(tc.tile_pool(name="ps", bufs=2, space="PSUM"))
        psp = ctx.enter_context(tc.tile_pool(name="psp", bufs=2, space="PSUM"))
        ps2 = ctx.enter_context(tc.tile_pool(name="ps2", bufs=2, space="PSUM"))

        # wqk + ones on the sync queue (ahead of the xT loads, so the first
        # qkT can start ASAP); the later-needed constants go via gpsimd so
        # they never delay the xT stream.
        # wqk split per c-tile so the very first qkT matmul only waits for
        # one quarter of it; later-needed constants go via gpsimd (ebt
        # first: scores consume it before the now-deferred v phase).
        wqk_s = const.tile([128, 4, 2 * DIM], BF16)
        wqk_r = wqk.ap().rearrange("(t p) f -> p t f", p=128)
        # wqk rides the scalar-hosted queue in parallel with xt on sync;
        # pair 0 below runs ct-major so compute starts once ct0 arrives;
        # ct0 split in half so the first ft-group matmuls start sooner.
        nc.scalar.dma_start(wqk_s[:, 0, 0:DIM], wqk_r[:, 0, 0:DIM])
        nc.scalar.dma_start(wqk_s[:, 0, DIM:], wqk_r[:, 0, DIM:])
        for ct in range(1, 4):
            nc.scalar.dma_start(wqk_s[:, ct, :], wqk_r[:, ct, :])

        xt_ap0 = xt.ap()
        xt0 = p_xt.tile([128, 4, 2, N], BF16, tag="xt", name="xt_boot")
        for ct in range(4):
            nc.sync.dma_start(xt0[:, ct, 0, :],
                              xt_ap0[0, 128 * ct:128 * (ct + 1), :])
            nc.gpsimd.dma_start(xt0[:, ct, 1, :],
                                xt_ap0[1, 128 * ct:128 * (ct + 1), :])
        oc_s = const.tile([128, 2, 128], FP8)
        nc.gpsimd.dma_start(
            oc_s[:].rearrange("p a b -> p (a b)"), ones8.ap()
        )
        eb_s = const.tile([128, 2 * NH * N], BF16)
        nc.gpsimd.dma_start(eb_s[:], ebt.ap())
        wv_s = const.tile([128, 4, DIM], BF16)
        nc.gpsimd.dma_start(wv_s[:], wv.ap().rearrange("(t p) f -> p t f", p=128))
        wp_s = const.tile([128, 4, DIM], BF16)
        nc.gpsimd.dma_start(wp_s[:], wp.ap().rearrange("(t p) f -> p t f", p=128))
        br_s = const.tile([128, DIM], F32)
        nc.gpsimd.dma_start(br_s[:], brep.ap())

        # zero-padded K storage (3 persistent buffers rotated by pair):
        # slot s=0 holds [kE; 0], slot s=1 holds [0; kO]. Score matmuls can
        # then run as full 128-row tile configs against the unpadded q tile
        # (the other parity's q rows are killed by the zeros), eliminating
        # the ~106ns PE array drain on every 64<->128-row config switch.
        kpads = []
        for b in range(3):
            t = const.tile([128, 4, 2, 2, N], BF16, tag=f"kpad{b}",
                           name=f"kpad{b}")
            nc.vector.memset(t[64:128, :, :, 0, :], 0.0)
            nc.vector.memset(t[0:64, :, :, 1, :], 0.0)
            kpads.append(t)

        xt_ap = xt.ap()
        out_ap = out.ap()

        # pp column index for (head, ktile): per head-pair the layout is
        # (e_k0, e_k1, o_k0, o_k1); score matmuls are ISSUED interleaved
        # e_k0, o_k0, e_k1, o_k1 so adjacent matmuls hit disjoint PE row
        # groups (and rowsum/avT orderings hit disjoint col groups).
        def ppi(h, kt):
            return (h // 2) * 4 + (h % 2) * 2 + kt

        def load_xt(wp2):
            # load xT (channel-major) for both windows: [128, ct, win, tok]
            if wp2 == 0:
                return xt0  # loaded up front across two queues
            xt_s = p_xt.tile([128, 4, 2, N], BF16, tag="xt")
            for wl in range(2):
                nc.sync.dma_start(
                    xt_s[:, :, wl, :],
                    xt_ap[2 * wp2 + wl].rearrange("(t p) q -> p t q", p=128),
                )
            return xt_s

        def qk_phase(wp2, xt_s, kpad):
            # qkT[feat, (win tok)] batched over the window pair (N=512 keeps
            # LDWEIGHTS hidden behind the matmul)
            qk_s = p_qk.tile([128, 8, 2, N], BF16, tag="qk")
            if wp2 == 0:
                # boot pair runs ct-major with 8 concurrent PSUM groups
                # (borrowing the idle scores/rowsum pools) so the first
                # matmul only waits for the ct0 DMAs
                accs = [ps.tile([128, 512], F32, tag="ps", name=f"qb_{f}")
                        for f in range(2)]
                accs += [psp.tile([128, 512], F32, tag="scp", name=f"qb_{f}")
                         for f in range(2, 4)]
                accs += [ps2.tile([128, 512], F32, tag="rs2", name=f"qb_{f}")
                         for f in range(4, 6)]
                for ct in range(4):
                    for ft in range(6):
                        nc.tensor.matmul(
                            accs[ft][:],
                            wqk_s[:, ct, ft * 128:(ft + 1) * 128],
                            xt_s[:, ct, :, :],
                            start=(ct == 0),
                            stop=(ct == 3),
                        )
                for ft in range(6):
                    if ft < 4:
                        nc.scalar.activation(
                            qk_s[:, ft, :, :].rearrange("p a q -> p (a q)"),
                            accs[ft][:], COPY,
                        )
                    else:
                        hp = ft - 4
                        nc.scalar.activation(
                            kpad[0:64, hp, :, 0, :],
                            accs[ft][0:64, :].rearrange(
                                "p (a q) -> p a q", a=2), COPY,
                        )
                        nc.vector.tensor_copy(
                            kpad[64:128, hp, :, 1, :],
                            accs[ft][64:128, :].rearrange(
                                "p (a q) -> p a q", a=2),
                        )
                for ft in range(6, 8):
                    acc = ps.tile([128, 512], F32, tag="ps")
                    for ct in range(4):
                        nc.tensor.matmul(
                            acc[:],
                            wqk_s[:, ct, ft * 128:(ft + 1) * 128],
                            xt_s[:, ct, :, :],
                            start=(ct == 0),
                            stop=(ct == 3),
                        )
                    hp = ft - 4
                    nc.scalar.activation(
                        kpad[0:64, hp, :, 0, :],
                        acc[0:64, :].rearrange("p (a q) -> p a q", a=2), COPY,
                    )
                    nc.vector.tensor_copy(
                        kpad[64:128, hp, :, 1, :],
                        acc[64:128, :].rearrange("p (a q) -> p a q", a=2),
                    )
            else:
                for ft in range(8):
                    acc = ps.tile([128, 512], F32, tag="ps")
                    for ct in range(4):
                        nc.tensor.matmul(
                            acc[:],
                            wqk_s[:, ct, ft * 128:(ft + 1) * 128],
                            xt_s[:, ct, :, :],
                            start=(ct == 0),
                            stop=(ct == 3),
                        )
                    if ft < 4:
                        # alternate copy engines so the 2-bank qk rotation
                        # recycles faster
                        dst = qk_s[:, ft, :, :].rearrange("p a q -> p (a q)")
                        if ft % 2 == 0:
                            nc.scalar.activation(dst, acc[:], COPY)
                        else:
                            nc.vector.tensor_copy(dst, acc[:])
                    else:
                        # k-ft: split into the zero-padded parity slots,
                        # halves run concurrently on ACT and DVE
                        hp = ft - 4
                        srcE = acc[0:64, :].rearrange(
                            "p (a q) -> p a q", a=2)
                        srcO = acc[64:128, :].rearrange(
                            "p (a q) -> p a q", a=2)
                        if ft % 2 == 0:
                            nc.scalar.activation(
                                kpad[0:64, hp, :, 0, :], srcE, COPY)
                            nc.vector.tensor_copy(
                                kpad[64:128, hp, :, 1, :], srcO)
                        else:
                            nc.vector.tensor_copy(
                                kpad[0:64, hp, :, 0, :], srcE)
                            nc.scalar.activation(
                                kpad[64:128, hp, :, 1, :], srcO, COPY)
            return qk_s

        def phase_v(xt_s):
            # v[tok, feat] (token-major), per window
            v_s = p_v.tile([128, 2, 2, DIM], BF16, tag="v")
            for wi in range(2):
                for kt in range(2):
                    acc = ps.tile([128, 512], F32, tag="ps")
                    for ct in range(4):
                        nc.tensor.matmul(
                            acc[:],
                            xt_s[:, ct, wi, kt * 128:(kt + 1) * 128],
                            wv_s[:, ct, :],
                            start=(ct == 0),
                            stop=(ct == 3),
                        )
                    nc.vector.tensor_copy(
                        v_s[:, wi, kt, :], acc[:]
                    )
            return v_s

        def phase_scores(qk_s, kpad, wi):
                # scores^T -> exp(scale*s) * exp(bias) -> pp; then rowsums
                pp_s = p_p.tile([128, 2 * NH * N], BF16, tag="pp",
                                name=f"pp_{wi}")
                # fp8 copy of pp (x 1/8) for the DoubleRow rowsum matmuls;
                # produced chunk-by-chunk on the idle gpsimd engine
                pp8_s = p_p8.tile([128, 2 * NH * N], FP8, tag="pp8",
                                  name=f"pp8_{wi}")
                for hp in range(4):
                    scpE = psp.tile([128, 2, N], F32, tag="scp")
                    scpO = psp.tile([128, 2, N], F32, tag="scp")
                    scp = [scpE, scpO]
                    for kt in range(2):
                        for s in range(2):
                            # full 128-row tile config: lhsT is the
                            # zero-padded parity slot, rhs the full q tile
                            nc.tensor.matmul(
                                scp[s][:, kt, :],
                                kpad[:, hp, wi, s,
                                     kt * 128:(kt + 1) * 128],
                                qk_s[:, hp, wi, :],
                                start=True,
                                stop=True,
                            )
                    te = p_e.tile([128, 2, 2 * N], BF16, tag="te",
                                  name=f"te_{wi}_{hp}")
                    for s in range(2):
                        nc.scalar.activation(
                            te[:, s, :],
                            scp[s][:].rearrange("p a q -> p (a q)"),
                            EXP, scale=SCALE,
                        )
                    off = hp * 4 * N
                    nc.vector.tensor_mul(
                        pp_s[:, off:off + 4 * N],
                        te[:].rearrange("p a q -> p (a q)"),
                        eb_s[:, off:off + 4 * N],
                    )
                    c0, c1 = hp * 4 * N, (hp + 1) * 4 * N
                    # pp8 is stored [p, kt, ap2, s, sub2, q] so each rowsum
                    # DoubleRow rhs [p, s, (sub2 q)] is one CONTIGUOUS 4N
                    # slice (strided matmul rhs APs are very slow); chunks
                    # alternate DVE/ACT (gpsimd is far too slow and its ops
                    # lock the shared SBUF port)
                    dst = pp8_s[:].rearrange(
                        "p (k a s b q) -> p k a s b q",
                        k=2, a=2, s=2, b=2)[:, :, hp // 2, :, hp % 2]
                    src = pp_s[:, c0:c1].rearrange(
                        "p (s k q) -> p k s q", s=2, k=2)
                    if hp % 2 == 0:
                        nc.vector.tensor_scalar_mul(dst, src, 0.03125)
                    else:
                        nc.scalar.activation(dst, src, COPY, scale=0.03125)
                # rowsums broadcast straight to feature rows: fp8 DoubleRow
                # ones-block matmuls. The packing dim carries the head
                # parity s (not kt): lhsT[:,0,m]=32 for m<64, lhsT[:,1,m]=32
                # for m>=64, so one instruction writes both heads' sums on
                # their own 64-partition groups (dst partition 0 -- the ISA
                # rejects DoubleRow dst offsets). Halves rowsum PE cycles.
                return pp_s, pp8_s

        def phase_rowsum(wi, pp8_s):
                rs0 = ps2.tile([128, 2, N], F32, tag="rs2", name=f"rs0_{wi}")
                rs1 = ps2.tile([128, 2, N], F32, tag="rs2", name=f"rs1_{wi}")
                rsp = [rs0, rs1]
                for ap2 in range(2):
                    for kt in range(2):
                        # 512-col DoubleRow (both sub2 blocks per inst);
                        # rhs is a contiguous 4N slice of pp8
                        base = (kt * 2 + ap2) * 4 * N
                        nc.tensor.matmul(
                            rsp[ap2][:, :, :],
                            oc_s[:],
                            pp8_s[:, base:base + 4 * N].rearrange(
                                "p (s m) -> p s m", s=2),
                            start=(kt == 0),
                            stop=(kt == 1),
                            perf_mode=DR,
                        )
                return rsp

        def phase_recip(wi, rsp):
                # reciprocal (fp32 fast-approx) per feature-tile pair;
                # result is directly the avT normalizer
                rb0 = p_rs.tile([128, 512], F32, tag="rb", name=f"rb0_{wi}")
                rb1 = p_rs.tile([128, 512], F32, tag="rb", name=f"rb1_{wi}")
                rbs = [rb0, rb1]
                for ap2 in range(2):
                    nc.vector.reciprocal_approx_fast(
                        rbs[ap2][:], rsp[ap2][:].rearrange("p a q -> p (a q)")
                    )
                return rbs

        def phase_av(v_s, wi, pp_s, rbs):
                # avT[f, q]: head pairs in PE column groups; batched
                # normalize-mult per two feature tiles
                av_s = p_av.tile([128, 4 * N], BF16, tag="av",
                                 name=f"av_{wi}")
                for ap2 in range(2):
                    aa = ps.tile([128, 512], F32, tag="work",
                                 name=f"aa_{wi}_{ap2}")
                    for sub2 in range(2):
                        ftl = 2 * ap2 + sub2
                        for sub in range(2):
                            h = 2 * ftl + sub
                            for kt in range(2):
                                nc.tensor.matmul(
                                    aa[sub * HD:(sub + 1) * HD,
                                       sub2 * N:(sub2 + 1) * N],
                                    v_s[:, wi, kt, h * HD:(h + 1) * HD],
                                    pp_s[:, ppi(h, kt) * N:(ppi(h, kt) + 1) * N],
                                    start=(kt == 0),
                                    stop=(kt == 1),
                                    tile_position=(0, sub * HD),
                                )
                    nc.vector.tensor_mul(
                        av_s[:, ap2 * 512:(ap2 + 1) * 512],
                        aa[:],
                        rbs[ap2][:],
                    )
                return av_s

        def phase_proj(w0, wi, av_s):
                # projection; add b_proj during PSUM->SBUF copy; output DMA
                # on the gpsimd queue (keeps the sync queue free for loads)
                w = w0 + wi
                for qt in range(2):
                    oo = ps.tile([128, 512], F32, tag="work",
                                 name=f"oo_{wi}_{qt}")
                    for ftl in range(4):
                        nc.tensor.matmul(
                            oo[:],
                            av_s[:, ftl * N + qt * 128:ftl * N + qt * 128 + 128],
                            wp_s[:, ftl, :],
                            start=(ftl == 0),
                            stop=(ftl == 3),
                        )
                    o_s = p_o.tile([128, DIM], BF16, tag="os",
                                   name=f"os_{wi}_{qt}")
                    if b_zero and qt == 0:
                        nc.scalar.activation(o_s[:], oo[:], COPY)
                    elif b_zero:
                        nc.vector.tensor_copy(o_s[:], oo[:])
                    else:
                        nc.vector.tensor_add(o_s[:], oo[:], br_s[:])
                    nc.gpsimd.dma_start(
                        out_ap[w, qt * 128:(qt + 1) * 128, :], o_s[:]
                    )

        def pair_tail(st, v_filler_xt):
            # the full post-qk chain for pair `st`, with the NEXT pair's
            # independent V matmuls emitted between the score matmuls and
            # the rowsums to cover the exp->mul->pp8 latency with PE work
            pp0, pp8_0 = phase_scores(st["qk_s"], st["kpad"], 0)
            pp1, pp8_1 = phase_scores(st["qk_s"], st["kpad"], 1)
            v_next = phase_v(v_filler_xt) if v_filler_xt is not None else None
            rsp0 = phase_rowsum(0, pp8_0)
            rsp1 = phase_rowsum(1, pp8_1)
            rb0 = phase_recip(0, rsp0)
            av0 = phase_av(st["v_s"], 0, pp0, rb0)
            phase_proj(st["w0"], 0, av0)
            rb1 = phase_recip(1, rsp1)
            av1 = phase_av(st["v_s"], 1, pp1, rb1)
            phase_proj(st["w0"], 1, av1)
            return v_next

        # software pipeline: emit pair k+1's qk block BEFORE pair k's
        # scores/av/proj tail so the PE runs long same-tile-config blocks
        # (each 64-row/128-row config switch costs a ~106ns array drain)
        prev = None
        for wp2 in range(w_count // 2):
            xt_s = load_xt(wp2)
            kpad = kpads[wp2 % 3]
            qk_s = qk_phase(wp2, xt_s, kpad)
            if prev is None:
                v_s = phase_v(xt_s)
                prev = {"w0": 2 * wp2, "qk_s": qk_s, "v_s": v_s,
                        "kpad": kpad}
                continue
            v_s = pair_tail(prev, xt_s)
            prev = {"w0": 2 * wp2, "qk_s": qk_s, "v_s": v_s,
                    "kpad": kpad}
        pair_tail(prev, None)

    nc.finalize()
    return nc


def _ones8_block():
    # [128, s(2), m(128)]: s=0 selects out rows 0-63 (even head), s=1 rows
    # 64-127 (odd head); value 32 cancels the 1/32 pp8 scale.
    o = np.zeros((128, 2, 128), np.float32)
    o[:, 0, :HD] = 32.0
    o[:, 1, HD:] = 32.0
    return np.ascontiguousarray(o.reshape(128, 256).astype(NPF8))


_NC_CACHE = {}


def _get_nc(w_count, b_zero):
    key = (w_count, b_zero)
    if key not in _NC_CACHE:
        _NC_CACHE[key] = build(w_count, b_zero)
    return _NC_CACHE[key]


def _prep(inputs, w_count):
    x = np.asarray(inputs["x"], dtype=np.float32)
    noise = np.asarray(inputs["noise"], dtype=np.float32)
    ns = np.asarray(inputs["noise_strength"], dtype=np.float32)
    wqkv = np.asarray(inputs["w_qkv"], dtype=np.float32)
    wproj = np.asarray(inputs["w_proj"], dtype=np.float32)
    bproj = np.asarray(inputs["b_proj"], dtype=np.float32)
    bt = np.asarray(inputs["bias_table"], dtype=np.float32)
    ri = np.asarray(inputs["rel_index"])

    xe = x + noise * ns                                     # [BS, N, DIM]
    xt = np.ascontiguousarray(xe.transpose(0, 2, 1).astype(NPBF))
    eb = np.exp(bt[ri])                                     # [q, k, h]
    ebT = eb.transpose(2, 1, 0)                             # [h, k, q]
    ebt = np.zeros((128, 2 * NH, N), np.float32)
    for h in range(NH):
        for kt in range(2):
            i = (h // 2) * 4 + (h % 2) * 2 + kt
            ebt[:, i, :] = ebT[h, kt * 128:(kt + 1) * 128, :]
    ebt = np.ascontiguousarray(ebt.reshape(128, 2 * NH * N).astype(NPBF))
    common = {
        "wqk": np.ascontiguousarray(wqkv[:, : 2 * DIM].astype(NPBF)),
        "wv": np.ascontiguousarray(wqkv[:, 2 * DIM:].astype(NPBF)),
        "wp": np.ascontiguousarray(wproj.astype(NPBF)),
        "brep": np.ascontiguousarray(
            np.broadcast_to(bproj.reshape(1, DIM), (128, DIM)).astype(np.float32)
        ),
        "ebt": ebt,
        "ones8": _ones8_block(),
    }
    in_maps = []
    for i in range(NCORES):
        m = dict(common)
        m["xt"] = np.ascontiguousarray(xt[i * w_count:(i + 1) * w_count])
        in_maps.append(m)
    return in_maps


def _run(inputs, w_count=W, trace=False, trace_cores=None):
    b_zero = not np.any(np.asarray(inputs["b_proj"], dtype=np.float32))
    nc = _get_nc(w_count, b_zero)
    in_maps = _prep(inputs, w_count)
    kw = {}
    if trace_cores is not None:
        kw["trace_cores"] = trace_cores
    res = run_bass_kernel_spmd(
        nc, in_maps, core_ids=list(range(NCORES)), trace=trace, **kw
    )
    full = np.concatenate(
        [res.results[i]["out"].astype(np.float32) for i in range(NCORES)], axis=0
    )
    return full, res


def kernel(**inputs):
    out, _ = _run(inputs, W, trace=False)
    return out


def kernel_profiled(inputs, w_count=W, trace_cores=None):
    out, res = _run(inputs, w_count, trace=True, trace_cores=trace_cores)
    return out, res



# revision 57
# speedup vs baseline: 1.1114x; 1.0007x over previous
"""Windowed (Swin-style) multi-head attention on 8 TRN2 NeuronCores.

Data-parallel: 256 independent windows -> 32 per core. Per window:
  qkv = x @ w_qkv ; per-head attn = softmax(q k^T * scale + bias) ; out = (attn v) @ w_proj + b_proj

Device-side layout strategy (all matmuls contract over the partition dim):
  - host pre-transposes x to channel-major xT[c, tok] so qT/kT are produced
    feature-major (ready to be score-matmul operands) and v token-major.
  - scores are computed TRANSPOSED, S^T[k, q] (lhsT = kT tile, rhs = qT), so
    softmax normalization runs over the partition axis:
      exp via ScalarE (scale folded in), * exp(bias) via VectorE,
      column-sums via ones-block matmul on TensorE (32 replicated rows at
      32-aligned partitions), reciprocal_approx_fast on VectorE, broadcast
      back to feature rows via an indicator matmul.
  - avT[f, q] = v-contracted matmul accumulated over k tiles; normalized
    avT is exactly the lhsT the projection matmul needs. b_proj is added
    (pre-broadcast on host) during the PSUM->SBUF output copy.
Matmul operands are bf16 (full-rate PE, fp32 PSUM accumulation); inputs are
rounded to bf16 on the host so they can be DMA'd directly.
"""

import sys

for _p in ("/opt/trn_rl_repo",):
    if _p not in sys.path:
        sys.path.insert(0, _p)

import ml_dtypes
import numpy as np
from contextlib import ExitStack

import concourse.bass as bass
import concourse.bacc as bacc
import concourse.mybir as mybir
from concourse import tile
from concourse.bass_utils import run_bass_kernel_spmd

NCORES = 8
BS = 256
W = BS // NCORES  # windows per core
N = 256           # tokens per window
DIM = 512
NH = 8
HD = 64
SCALE = HD ** -0.5
F32 = mybir.dt.float32
BF16 = mybir.dt.bfloat16
FP8 = mybir.dt.float8e4
NPBF = ml_dtypes.bfloat16
NPF8 = ml_dtypes.float8_e4m3fn
EXP = mybir.ActivationFunctionType.Exp
COPY = mybir.ActivationFunctionType.Copy
DR = mybir.MatmulPerfMode.DoubleRow


def build(w_count=W, b_zero=False):
    nc = bacc.Bacc(None, target_bir_lowering=False)
    xt = nc.declare_dram_parameter("xt", [w_count, DIM, N], BF16, False)
    wqk = nc.declare_dram_parameter("wqk", [DIM, 2 * DIM], BF16, False)
    wv = nc.declare_dram_parameter("wv", [DIM, DIM], BF16, False)
    wp = nc.declare_dram_parameter("wp", [DIM, DIM], BF16, False)
    brep = nc.declare_dram_parameter("brep", [128, DIM], F32, False)
    ebt = nc.declare_dram_parameter("ebt", [128, 2 * NH * N], BF16, False)
    ones8 = nc.declare_dram_parameter("ones8", [128, 2 * 128], FP8, False)
    out = nc.declare_dram_parameter("out", [w_count, N, DIM], BF16, True)

    with ExitStack() as ctx:
        tc = ctx.enter_context(tile.TileContext(nc))
        const = ctx.enter_context(tc.tile_pool(name="const", bufs=1))
        p_xt = ctx.enter_context(tc.tile_pool(name="xt", bufs=4))
        p_qk = ctx.enter_context(tc.tile_pool(name="qk", bufs=4))
        p_v = ctx.enter_context(tc.tile_pool(name="v", bufs=3))
        p_p = ctx.enter_context(tc.tile_pool(name="pp", bufs=4))
        p_p8 = ctx.enter_context(tc.tile_pool(name="pp8", bufs=4))
        p_e = ctx.enter_context(tc.tile_pool(name="te", bufs=10))
        p_bc = ctx.enter_context(tc.tile_pool(name="bc", bufs=4))
        p_av = ctx.enter_context(tc.tile_pool(name="av", bufs=4))
        p_rs = ctx.enter_context(tc.tile_pool(name="rs", bufs=3))
        p_o = ctx.enter_context(tc.tile_pool(name="os", bufs=6))
        ps = ctx.enter_context(tc.tile_pool(name="ps", bufs=2, space="PSUM"))
        psp = ctx.enter_context(tc.tile_pool(name="psp", bufs=2, space="PSUM"))
        ps2 = ctx.enter_context(tc.tile_pool(name="ps2", bufs=2, space="PSUM"))

        # wqk + ones on the sync queue (ahead of the xT loads, so the first
        # qkT can start ASAP); the later-needed constants go via gpsimd so
        # they never delay the xT stream.
        # wqk split per c-tile so the very first qkT matmul only waits for
        # one quarter of it; later-needed constants go via gpsimd (ebt
        # first: scores consume it before the now-deferred v phase).
        wqk_s = const.tile([128, 4, 2 * DIM], BF16)
        wqk_r = wqk.ap().rearrange("(t p) f -> p t f", p=128)
        # wqk rides the scalar-hosted queue in parallel with xt on sync;
        # pair 0 below runs ct-major so compute starts once ct0 arrives;
        # ct0 split in half so the first ft-group matmuls start sooner.
        nc.scalar.dma_start(wqk_s[:, 0, 0:DIM], wqk_r[:, 0, 0:DIM])
        nc.scalar.dma_start(wqk_s[:, 0, DIM:], wqk_r[:, 0, DIM:])
        for ct in range(1, 4):
            nc.scalar.dma_start(wqk_s[:, ct, :], wqk_r[:, ct, :])

        xt_ap0 = xt.ap()
        xt0 = p_xt.tile([128, 4, 2, N], BF16, tag="xt", name="xt_boot")
        for ct in range(4):
            nc.sync.dma_start(xt0[:, ct, 0, :],
                              xt_ap0[0, 128 * ct:128 * (ct + 1), :])
            nc.gpsimd.dma_start(xt0[:, ct, 1, :],
                                xt_ap0[1, 128 * ct:128 * (ct + 1), :])
        oc_s = const.tile([128, 2, 128], FP8)
        nc.gpsimd.dma_start(
            oc_s[:].rearrange("p a b -> p (a b)"), ones8.ap()
        )
        eb_s = const.tile([128, 2 * NH * N], BF16)
        nc.gpsimd.dma_start(eb_s[:], ebt.ap())
        wv_s = const.tile([128, 4, DIM], BF16)
        nc.gpsimd.dma_start(wv_s[:], wv.ap().rearrange("(t p) f -> p t f", p=128))
        wp_s = const.tile([128, 4, DIM], BF16)
        nc.gpsimd.dma_start(wp_s[:], wp.ap().rearrange("(t p) f -> p t f", p=128))
        br_s = const.tile([128, DIM], F32)
        nc.gpsimd.dma_start(br_s[:], brep.ap())

        # zero-padded K storage (3 persistent buffers rotated by pair):
        # slot s=0 holds [kE; 0], slot s=1 holds [0; kO]. Score matmuls can
        # then run as full 128-row tile configs against the unpadded q tile
        # (the other parity's q rows are killed by the zeros), eliminating
        # the ~106ns PE array drain on every 64<->128-row config switch.
        kpads = []
        for b in range(3):
            t = const.tile([128, 4, 2, 2, N], BF16, tag=f"kpad{b}",
                           name=f"kpad{b}")
            nc.vector.memset(t[64:128, :, :, 0, :], 0.0)
            nc.vector.memset(t[0:64, :, :, 1, :], 0.0)
            kpads.append(t)

        xt_ap = xt.ap()
        out_ap = out.ap()

        # pp column index for (head, ktile): per head-pair the layout is
        # (e_k0, e_k1, o_k0, o_k1); score matmuls are ISSUED interleaved
        # e_k0, o_k0, e_k1, o_k1 so adjacent matmuls hit disjoint PE row
        # groups (and rowsum/avT orderings hit disjoint col groups).
        def ppi(h, kt):
            return (h // 2) * 4 + (h % 2) * 2 + kt

        def load_xt(wp2):
            # load xT (channel-major) for both windows: [128, ct, win, tok]
            if wp2 == 0:
                return xt0  # loaded up front across two queues
            xt_s = p_xt.tile([128, 4, 2, N], BF16, tag="xt")
            for wl in range(2):
                nc.sync.dma_start(
                    xt_s[:, :, wl, :],
                    xt_ap[2 * wp2 + wl].rearrange("(t p) q -> p t q", p=128),
                )
            return xt_s

        def qk_phase(wp2, xt_s, kpad):
            # qkT[feat, (win tok)] batched over the window pair (N=512 keeps
            # LDWEIGHTS hidden behind the matmul)
            qk_s = p_qk.tile([128, 8, 2, N], BF16, tag="qk")
            if wp2 == 0:
                # boot pair runs ct-major with 8 concurrent PSUM groups
                # (borrowing the idle scores/rowsum pools) so the first
                # matmul only waits for the ct0 DMAs
                accs = [ps.tile([128, 512], F32, tag="ps", name=f"qb_{f}")
                        for f in range(2)]
                accs += [psp.tile([128, 512], F32, tag="scp", name=f"qb_{f}")
                         for f in range(2, 4)]
                accs += [ps2.tile([128, 512], F32, tag="rs2", name=f"qb_{f}")
                         for f in range(4, 6)]
                for ct in range(4):
                    for ft in range(6):
                        nc.tensor.matmul(
                            accs[ft][:],
                            wqk_s[:, ct, ft * 128:(ft + 1) * 128],
                            xt_s[:, ct, :, :],
                            start=(ct == 0),
                            stop=(ct == 3),
                        )
                for ft in range(6):
                    if ft < 4:
                        nc.scalar.activation(
                            qk_s[:, ft, :, :].rearrange("p a q -> p (a q)"),
                            accs[ft][:], COPY,
                        )
                    else:
                        hp = ft - 4
                        nc.scalar.activation(
                            kpad[0:64, hp, :, 0, :],
                            accs[ft][0:64, :].rearrange(
                                "p (a q) -> p a q", a=2), COPY,
                        )
                        nc.vector.tensor_copy(
                            kpad[64:128, hp, :, 1, :],
                            accs[ft][64:128, :].rearrange(
                                "p (a q) -> p a q", a=2),
                        )
                for ft in range(6, 8):
                    acc = ps.tile([128, 512], F32, tag="ps")
                    for ct in range(4):
                        nc.tensor.matmul(
                            acc[:],
                            wqk_s[:, ct, ft * 128:(ft + 1) * 128],
                            xt_s[:, ct, :, :],
                            start=(ct == 0),
                            stop=(ct == 3),
                        )
                    hp = ft - 4
                    nc.scalar.activation(
                        kpad[0:64, hp, :, 0, :],
                        acc[0:64, :].rearrange("p (a q) -> p a q", a=2), COPY,
                    )
                    nc.vector.tensor_copy(
                        kpad[64:128, hp, :, 1, :],
                        acc[64:128, :].rearrange("p (a q) -> p a q", a=2),
                    )
            else:
                for ft in range(8):
                    acc = ps.tile([128, 512], F32, tag="ps")
                    for ct in range(4):
                        nc.tensor.matmul(
                            acc[:],
                            wqk_s[:, ct, ft * 128:(ft + 1) * 128],
                            xt_s[:, ct, :, :],
                            start=(ct == 0),
                            stop=(ct == 3),
                        )
                    if ft < 4:
                        # alternate copy engines so the 2-bank qk rotation
                        # recycles faster
                        dst = qk_s[:, ft, :, :].rearrange("p a q -> p (a q)")
                        if ft % 2 == 0:
                            nc.scalar.activation(dst, acc[:], COPY)
                        else:
                            nc.vector.tensor_copy(dst, acc[:])
                    else:
                        # k-ft: split into the zero-padded parity slots,
                        # halves run concurrently on ACT and DVE
                        hp = ft - 4
                        srcE = acc[0:64, :].rearrange(
                            "p (a q) -> p a q", a=2)
                        srcO = acc[64:128, :].rearrange(
                            "p (a q) -> p a q", a=2)
                        if ft % 2 == 0:
                            nc.scalar.activation(
                                kpad[0:64, hp, :, 0, :], srcE, COPY)
                            nc.vector.tensor_copy(
                                kpad[64:128, hp, :, 1, :], srcO)
                        else:
                            nc.vector.tensor_copy(
                                kpad[0:64, hp, :, 0, :], srcE)
                            nc.scalar.activation(
                                kpad[64:128, hp, :, 1, :], srcO, COPY)
            return qk_s

        def phase_v(xt_s):
            # v[tok, feat] (token-major), per window
            v_s = p_v.tile([128, 2, 2, DIM], BF16, tag="v")
            for wi in range(2):
                for kt in range(2):
                    acc = ps.tile([128, 512], F32, tag="ps")
                    for ct in range(4):
                        nc.tensor.matmul(
                            acc[:],
                            xt_s[:, ct, wi, kt * 128:(kt + 1) * 128],
                            wv_s[:, ct, :],
                            start=(ct == 0),
                            stop=(ct == 3),
                        )
                    nc.vector.tensor_copy(
                        v_s[:, wi, kt, :], acc[:]
                    )
            return v_s

        def phase_scores(qk_s, kpad, wi):
                # scores^T -> exp(scale*s) * exp(bias) -> pp; then rowsums
                pp_s = p_p.tile([128, 2 * NH * N], BF16, tag="pp",
                                name=f"pp_{wi}")
                # fp8 copy of pp (x 1/8) for the DoubleRow rowsum matmuls;
                # produced chunk-by-chunk on the idle gpsimd engine
                pp8_s = p_p8.tile([128, 2 * NH * N], FP8, tag="pp8",
                                  name=f"pp8_{wi}")
                for hp in range(4):
                    scpE = psp.tile([128, 2, N], F32, tag="scp")
                    scpO = psp.tile([128, 2, N], F32, tag="scp")
                    scp = [scpE, scpO]
                    for kt in range(2):
                        for s in range(2):
                            # full 128-row tile config: lhsT is the
                            # zero-padded parity slot, rhs the full q tile
                            nc.tensor.matmul(
                                scp[s][:, kt, :],
                                kpad[:, hp, wi, s,
                                     kt * 128:(kt + 1) * 128],
                                qk_s[:, hp, wi, :],
                                start=True,
                                stop=True,
                            )
                    te = p_e.tile([128, 2, 2 * N], BF16, tag="te",
                                  name=f"te_{wi}_{hp}")
                    for s in range(2):
                        nc.scalar.activation(
                            te[:, s, :],
                            scp[s][:].rearrange("p a q -> p (a q)"),
                            EXP, scale=SCALE,
                        )
                    off = hp * 4 * N
                    nc.vector.tensor_mul(
                        pp_s[:, off:off + 4 * N],
                        te[:].rearrange("p a q -> p (a q)"),
                        eb_s[:, off:off + 4 * N],
                    )
                    c0, c1 = hp * 4 * N, (hp + 1) * 4 * N
                    # pp8 is stored [p, kt, ap2, s, sub2, q] so each rowsum
                    # DoubleRow rhs [p, s, (sub2 q)] is one CONTIGUOUS 4N
                    # slice (strided matmul rhs APs are very slow); chunks
                    # alternate DVE/ACT (gpsimd is far too slow and its ops
                    # lock the shared SBUF port)
                    dst = pp8_s[:].rearrange(
                        "p (k a s b q) -> p k a s b q",
                        k=2, a=2, s=2, b=2)[:, :, hp // 2, :, hp % 2]
                    src = pp_s[:, c0:c1].rearrange(
                        "p (s k q) -> p k s q", s=2, k=2)
                    if hp % 2 == 0:
                        nc.vector.tensor_scalar_mul(dst, src, 0.03125)
                    else:
                        nc.scalar.activation(dst, src, COPY, scale=0.03125)
                # rowsums broadcast straight to feature rows: fp8 DoubleRow
                # ones-block matmuls. The packing dim carries the head
                # parity s (not kt): lhsT[:,0,m]=32 for m<64, lhsT[:,1,m]=32
                # for m>=64, so one instruction writes both heads' sums on
                # their own 64-partition groups (dst partition 0 -- the ISA
                # rejects DoubleRow dst offsets). Halves rowsum PE cycles.
                return pp_s, pp8_s

        def phase_rowsum(wi, pp8_s):
                rs0 = ps2.tile([128, 2, N], F32, tag="rs2", name=f"rs0_{wi}")
                rs1 = ps2.tile([128, 2, N], F32, tag="rs2", name=f"rs1_{wi}")
                rsp = [rs0, rs1]
                for ap2 in range(2):
                    for kt in range(2):
                        # 512-col DoubleRow (both sub2 blocks per inst);
                        # rhs is a contiguous 4N slice of pp8
                        base = (kt * 2 + ap2) * 4 * N
                        nc.tensor.matmul(
                            rsp[ap2][:, :, :],
                            oc_s[:],
                            pp8_s[:, base:base + 4 * N].rearrange(
                                "p (s m) -> p s m", s=2),
                            start=(kt == 0),
                            stop=(kt == 1),
                            perf_mode=DR,
                        )
                return rsp

        def phase_recip(wi, rsp):
                # reciprocal (fp32 fast-approx) per feature-tile pair;
                # result is directly the avT normalizer
                rb0 = p_rs.tile([128, 512], F32, tag="rb", name=f"rb0_{wi}")
                rb1 = p_rs.tile([128, 512], F32, tag="rb", name=f"rb1_{wi}")
                rbs = [rb0, rb1]
                for ap2 in range(2):
                    nc.vector.reciprocal_approx_fast(
                        rbs[ap2][:], rsp[ap2][:].rearrange("p a q -> p (a q)")
                    )
                return rbs

        def phase_av(v_s, wi, pp_s, rbs):
                # avT[f, q]: head pairs in PE column groups; batched
                # normalize-mult per two feature tiles
                av_s = p_av.tile([128, 4 * N], BF16, tag="av",
                                 name=f"av_{wi}")
                for ap2 in range(2):
                    aa = ps.tile([128, 512], F32, tag="work",
                                 name=f"aa_{wi}_{ap2}")
                    for sub2 in range(2):
                        ftl = 2 * ap2 + sub2
                        for sub in range(2):
                            h = 2 * ftl + sub
                            for kt in range(2):
                                nc.tensor.matmul(
                                    aa[sub * HD:(sub + 1) * HD,
                                       sub2 * N:(sub2 + 1) * N],
                                    v_s[:, wi, kt, h * HD:(h + 1) * HD],
                                    pp_s[:, ppi(h, kt) * N:(ppi(h, kt) + 1) * N],
                                    start=(kt == 0),
                                    stop=(kt == 1),
                                    tile_position=(0, sub * HD),
                                )
                    nc.vector.tensor_mul(
                        av_s[:, ap2 * 512:(ap2 + 1) * 512],
                        aa[:],
                        rbs[ap2][:],
                    )
                return av_s

        def phase_proj(w0, wi, av_s):
                # projection; add b_proj during PSUM->SBUF copy; output DMA
                # on the gpsimd queue (keeps the sync queue free for loads)
                w = w0 + wi
                for qt in range(2):
                    oo = ps.tile([128, 512], F32, tag="work",
                                 name=f"oo_{wi}_{qt}")
                    for ftl in range(4):
                        nc.tensor.matmul(
                            oo[:],
                            av_s[:, ftl * N + qt * 128:ftl * N + qt * 128 + 128],
                            wp_s[:, ftl, :],
                            start=(ftl == 0),
                            stop=(ftl == 3),
                        )
                    o_s = p_o.tile([128, DIM], BF16, tag="os",
                                   name=f"os_{wi}_{qt}")
                    if b_zero and qt == 0:
                        nc.scalar.activation(o_s[:], oo[:], COPY)
                    elif b_zero:
                        nc.vector.tensor_copy(o_s[:], oo[:])
                    else:
                        nc.vector.tensor_add(o_s[:], oo[:], br_s[:])
                    nc.gpsimd.dma_start(
                        out_ap[w, qt * 128:(qt + 1) * 128, :], o_s[:]
                    )

        def pair_tail(st, v_filler_xt):
            # the full post-qk chain for pair `st`, with the NEXT pair's
            # independent V matmuls emitted between the score matmuls and
            # the rowsums to cover the exp->mul->pp8 latency with PE work
            pp0, pp8_0 = phase_scores(st["qk_s"], st["kpad"], 0)
            pp1, pp8_1 = phase_scores(st["qk_s"], st["kpad"], 1)
            v_next = phase_v(v_filler_xt) if v_filler_xt is not None else None
            rsp0 = phase_rowsum(0, pp8_0)
            rsp1 = phase_rowsum(1, pp8_1)
            rb0 = phase_recip(0, rsp0)
            av0 = phase_av(st["v_s"], 0, pp0, rb0)
            phase_proj(st["w0"], 0, av0)
            rb1 = phase_recip(1, rsp1)
            av1 = phase_av(st["v_s"], 1, pp1, rb1)
            phase_proj(st["w0"], 1, av1)
            return v_next

        # software pipeline: emit pair k+1's qk block BEFORE pair k's
        # scores/av/proj tail so the PE runs long same-tile-config blocks
        # (each 64-row/128-row config switch costs a ~106ns array drain)
        prev = None
        for wp2 in range(w_count // 2):
            xt_s = load_xt(wp2)
            kpad = kpads[wp2 % 3]
            qk_s = qk_phase(wp2, xt_s, kpad)
            if prev is None:
                v_s = phase_v(xt_s)
                prev = {"w0": 2 * wp2, "qk_s": qk_s, "v_s": v_s,
                        "kpad": kpad}
                continue
            v_s = pair_tail(prev, xt_s)
            prev = {"w0": 2 * wp2, "qk_s": qk_s, "v_s": v_s,
                    "kpad": kpad}
        pair_tail(prev, None)

    nc.finalize()
    return nc


def _ones8_block():
    # [128, s(2), m(128)]: s=0 selects out rows 0-63 (even head), s=1 rows
    # 64-127 (odd head); value 32 cancels the 1/32 pp8 scale.
    o = np.zeros((128, 2, 128), np.float32)
    o[:, 0, :HD] = 32.0
    o[:, 1, HD:] = 32.0
    return np.ascontiguousarray(o.reshape(128, 256).astype(NPF8))


_NC_CACHE = {}


def _get_nc(w_count, b_zero):
    key = (w_count, b_zero)
    if key not in _NC_CACHE:
        _NC_CACHE[key] = build(w_count, b_zero)
    return _NC_CACHE[key]


def _prep(inputs, w_count):
    x = np.asarray(inputs["x"], dtype=np.float32)
    noise = np.asarray(inputs["noise"], dtype=np.float32)
    ns = np.asarray(inputs["noise_strength"], dtype=np.float32)
    wqkv = np.asarray(inputs["w_qkv"], dtype=np.float32)
    wproj = np.asarray(inputs["w_proj"], dtype=np.float32)
    bproj = np.asarray(inputs["b_proj"], dtype=np.float32)
    bt = np.asarray(inputs["bias_table"], dtype=np.float32)
    ri = np.asarray(inputs["rel_index"])

    xe = x + noise * ns                                     # [BS, N, DIM]
    xt = np.ascontiguousarray(xe.transpose(0, 2, 1).astype(NPBF))
    eb = np.exp(bt[ri])                                     # [q, k, h]
    ebT = eb.transpose(2, 1, 0)                             # [h, k, q]
    ebt = np.zeros((128, 2 * NH, N), np.float32)
    for h in range(NH):
        for kt in range(2):
            i = (h // 2) * 4 + (h % 2) * 2 + kt
            ebt[:, i, :] = ebT[h, kt * 128:(kt + 1) * 128, :]
    ebt = np.ascontiguousarray(ebt.reshape(128, 2 * NH * N).astype(NPBF))
    common = {
        "wqk": np.ascontiguousarray(wqkv[:, : 2 * DIM].astype(NPBF)),
        "wv": np.ascontiguousarray(wqkv[:, 2 * DIM:].astype(NPBF)),
        "wp": np.ascontiguousarray(wproj.astype(NPBF)),
        "brep": np.ascontiguousarray(
            np.broadcast_to(bproj.reshape(1, DIM), (128, DIM)).astype(np.float32)
        ),
        "ebt": ebt,
        "ones8": _ones8_block(),
    }
    in_maps = []
    for i in range(NCORES):
        m = dict(common)
        m["xt"] = np.ascontiguousarray(xt[i * w_count:(i + 1) * w_count])
        in_maps.append(m)
    return in_maps


def _run(inputs, w_count=W, trace=False, trace_cores=None):
    b_zero = not np.any(np.asarray(inputs["b_proj"], dtype=np.float32))
    nc = _get_nc(w_count, b_zero)
    in_maps = _prep(inputs, w_count)
    kw = {}
    if trace_cores is not None:
        kw["trace_cores"] = trace_cores
    res = run_bass_kernel_spmd(
        nc, in_maps, core_ids=list(range(NCORES)), trace=trace, **kw
    )
    full = np.concatenate(
        [res.results[i]["out"].astype(np.float32) for i in range(NCORES)], axis=0
    )
    return full, res


def kernel(**inputs):
    out, _ = _run(inputs, W, trace=False)
    return out


def kernel_profiled(inputs, w_count=W, trace_cores=None):
    out, res = _run(inputs, w_count, trace=True, trace_cores=trace_cores)
    return out, res

